# revision 9
# baseline (speedup 1.0000x reference)
"""MoE (top-2 of 8 experts + shared expert, SwiGLU) on 8 trn2 NeuronCores.

Sharding: data-parallel over tokens (512/core), with SPARSE routed experts.
The host computes the routing (top-2 indices), the renormalized gates, and
the shared-expert mixing factor alpha = sigmoid(x@sg_w+b); it stages, per
core, a gathered activation matrix xg whose columns are the tokens assigned
to each expert (concatenated expert blocks, padded to a shared capacity
table) plus a small metadata block (slot->token index, per-slot combined
gate g*(1-alpha), per-token alpha).

Device schedule (per core, per iteration):
- A one-hot scatter matrix M[slot, token] is built on device (iota vs the
  staged slot->token index vector); g*(1-alpha) is folded into M's rows, so
  the scatter matmul directly produces (1-alpha)*routed.
- Routed experts run per expert chunk (SwiGLU, silu on the Act engine);
  shared-expert gate/up chunks are interleaved BETWEEN routed experts so
  the PE stays busy while routed weights stream from HBM (the routed phase
  alone is DMA-bound).
- Output phase per (dh, ti) tile: scatter matmuls accumulate
  r' = (1-alpha)*routed into one PSUM bank, shared down-projection
  accumulates s into another; out = alpha*s + r' via one Act copy-scale
  and one DVE add, then the store DMA issues immediately (on the Act DGE
  queue, so the SP DGE queue drains into the next iteration's prefetch).

Slot space is tiled in per-expert chunks of <=128 slots (rows always start
at partition 0: the hardware only supports 32-aligned start partitions).
"""

import numpy as np
from contextlib import ExitStack

import concourse.bass as bass
import concourse.mybir as mybir
import concourse.tile as tile
from concourse import bacc
from concourse.bass_utils import run_bass_kernel_spmd

B, S, D = 4, 1024, 768
E, H, HS = 8, 768, 3072
N_CORES = 8
T = (B * S) // N_CORES  # 512 tokens per core
P = 128
KD = D // P    # 6 k-tiles over d_model
MH = H // P    # 6 m-tiles over expert hidden
MS = HS // P   # 24 m-tiles over shared hidden
TM = T // P    # 4 token tiles
DH = D // 2    # 384, psum-bank-sized half of d_model
NSC = 12       # shared gate/up weight chunks (2 hs-tiles each)
F32 = mybir.dt.float32
BF16 = mybir.dt.bfloat16

Alu = mybir.AluOpType
Act = mybir.ActivationFunctionType
AX = mybir.AxisListType

_DEFAULT_CAPS = (153, 131, 144, 142, 140, 139, 138, 140)
_CAPS = _DEFAULT_CAPS

# after routed expert e, compute this many pending shared gate/up chunks
_SHARED_SCHED = (1, 2, 1, 2, 2, 1, 2, 1)
assert sum(_SHARED_SCHED) == NSC


def _grid(caps):
    """Slot layout: experts packed contiguously; L is a multiple of 128
    (host pads the last expert's capacity). Segments split each expert's
    span at global 128-slot tile boundaries, so every segment lives inside
    one 128-row tile: (expert, global_start, width)."""
    offs = np.concatenate([[0], np.cumsum(caps)]).astype(int)
    L = int(offs[-1])
    segs = []
    for e in range(E):
        s = int(offs[e])
        end = int(offs[e + 1])
        while s < end:
            w = min(end - s, P - (s % P), P)
            segs.append((e, s, w))
            s += w
    return offs, L, segs


def _build_program(repeat=1, caps=None, repeat_staggered=False, **_ignored):
    caps = tuple(caps) if caps is not None else _CAPS
    offs, L, chunks = _grid(caps)
    NCH = len(chunks)
    MW = max(caps)

    nc = bacc.Bacc("TRN2", target_bir_lowering=False, debug=False,
                   num_devices=N_CORES)

    xb = nc.dram_tensor("xb", [D, T], BF16, kind="ExternalInput")
    xg = nc.dram_tensor("xg", [D, L], BF16, kind="ExternalInput")
    # meta: [tidx (NCH) | gate*(1-alpha) (NCH) | alpha (TM)]
    meta = nc.dram_tensor("meta", [P, 2 * NCH + TM], F32,
                          kind="ExternalInput")
    wg = nc.dram_tensor("wg", [E, D, H], BF16, kind="ExternalInput")
    wu = nc.dram_tensor("wu", [E, D, H], BF16, kind="ExternalInput")
    wd = nc.dram_tensor("wd", [E, H, D], BF16, kind="ExternalInput")
    wsg = nc.dram_tensor("wsg", [D, HS], BF16, kind="ExternalInput")
    wsu = nc.dram_tensor("wsu", [D, HS], BF16, kind="ExternalInput")
    wsd = nc.dram_tensor("wsd", [HS, D], BF16, kind="ExternalInput")
    out = nc.dram_tensor("out", [T, D], F32, kind="ExternalOutput")

    wgr = wg.rearrange("e (k p) m -> e p k m", p=P)
    wur = wu.rearrange("e (k p) m -> e p k m", p=P)
    wdr = wd.rearrange("e (k p) m -> e p k m", p=P)
    wsgr = wsg.rearrange("(k p) (c m) -> c p k m", p=P, c=NSC)
    wsur = wsu.rearrange("(k p) (c m) -> c p k m", p=P, c=NSC)
    # wsd chunked by (jc: 4 groups of 6 hs-tiles) x (dh: 2 halves of d)
    wsdr = wsd.rearrange("(c j p) (h m) -> c h p j m", p=P, c=4, h=2)

    with tile.TileContext(nc) as tc:
        def emit_body(bi):
          with ExitStack() as ctx:
            # ---- early loads: xg k0 first, then expert-0 gate/up weights
            # (PE's critical path), then the rest of xg ----
            xgp = ctx.enter_context(tc.tile_pool(name=f"xg{bi}", bufs=KD))
            xgs = [xgp.tile([P, L], BF16, tag="xg", name=f"xg{bi}_{k}")
                   for k in range(KD)]
            nc.sync.dma_start(xgs[0][:], xg[0:P, :])
            smallp = ctx.enter_context(tc.tile_pool(name=f"small{bi}", bufs=2))
            smf = smallp.tile([P, 2 * NCH + TM], F32, tag="smf")
            tidx = smf[:, 0:NCH]
            g2 = smf[:, NCH:2 * NCH]
            alph = smf[:, 2 * NCH:2 * NCH + TM]

            iota_t = smallp.tile([P, T], F32, tag="iota")
            nc.gpsimd.iota(iota_t[:], [[1, T]], channel_multiplier=0,
                           allow_small_or_imprecise_dtypes=True)

            # ---- long-lived activation storage ----
            mgp = ctx.enter_context(tc.tile_pool(name=f"mg{bi}", bufs=NCH))
            ytp = ctx.enter_context(tc.tile_pool(name=f"yt{bi}", bufs=NCH))
            asp = ctx.enter_context(tc.tile_pool(name=f"as{bi}", bufs=MS))
            xbp = ctx.enter_context(tc.tile_pool(name=f"xb{bi}", bufs=KD))
            wsdp = ctx.enter_context(tc.tile_pool(name=f"wsd{bi}", bufs=6))

            mgs = [mgp.tile([P, T], BF16, tag="mg", name=f"mg{bi}_{q}")
                   for q in range(NCH)]
            yts = [ytp.tile([P, D], BF16, tag="yt", name=f"yt{bi}_{q}")
                   for q in range(NCH)]
            as_tiles = [asp.tile([P, T], BF16, tag="as", name=f"as{bi}_{j}")
                        for j in range(MS)]
            wsd_half = {}  # dh -> [4 tiles]

            with tc.tile_pool(name=f"wge{bi}", bufs=2) as wgep, \
                 tc.tile_pool(name=f"wue{bi}", bufs=2) as wuep, \
                 tc.tile_pool(name=f"wde{bi}", bufs=2) as wdep, \
                 tc.tile_pool(name=f"wsh{bi}", bufs=4) as wshp, \
                 tc.tile_pool(name=f"psum_g{bi}", bufs=2, space="PSUM") as pgp, \
                 tc.tile_pool(name=f"psum_u{bi}", bufs=2, space="PSUM") as pup, \
                 tc.tile_pool(name=f"psum_y{bi}", bufs=2, space="PSUM") as pyp, \
                 tc.tile_pool(name=f"sil{bi}", bufs=3) as silp, \
                 tc.tile_pool(name=f"a2{bi}", bufs=2 * MH + 2) as a2p:

                def load_gu(e):
                    wg_t = wgep.tile([P, KD, H], BF16, tag="wge")
                    nc.sync.dma_start(wg_t[:], wgr[e])
                    wu_t = wuep.tile([P, KD, H], BF16, tag="wue")
                    nc.sync.dma_start(wu_t[:], wur[e])
                    return wg_t, wu_t

                def load_d(e):
                    wd_t = wdep.tile([P, KD, D], BF16, tag="wde")
                    nc.sync.dma_start(wd_t[:], wdr[e])
                    return wd_t

                def load_shared(c):
                    wsg_t = wshp.tile([P, KD, 2 * P], BF16, tag="wsg")
                    nc.sync.dma_start(wsg_t[:], wsgr[c])
                    wsu_t = wshp.tile([P, KD, 2 * P], BF16, tag="wsu")
                    nc.sync.dma_start(wsu_t[:], wsur[c])
                    return wsg_t, wsu_t

                # initial prefetch: expert 0 g/u right behind xg k0, then
                # the remaining xg k-tiles, meta, xb, shared chunk 0, wd0
                gu_next = load_gu(0)
                for k in range(1, KD):
                    nc.sync.dma_start(xgs[k][:], xg[k * P:(k + 1) * P, :])
                nc.sync.dma_start(smf[:], meta[:, :])
                xbs = []
                for k in range(KD):
                    t_ = xbp.tile([P, T], BF16, tag="xb")
                    nc.sync.dma_start(t_[:], xb[k * P:(k + 1) * P, :])
                    xbs.append(t_)
                sh_w = [load_shared(0)]
                d_next = load_d(0)

                # ---- scatter matrix build (DVE/Act, runs during DMA) ----
                for q, (e, g0, w) in enumerate(chunks):
                    mf = silp.tile([P, T], BF16, tag="mf", name="mf")
                    nc.vector.tensor_scalar(mf[0:w], iota_t[0:w],
                                            tidx[0:w, q:q + 1], None,
                                            Alu.is_equal)
                    nc.scalar.activation(mgs[q][0:w], mf[0:w], Act.Copy,
                                         scale=g2[0:w, q:q + 1])

                a_tiles = {}

                def gate_up(e, wg_t, wu_t):
                    c = caps[e]
                    o = int(offs[e])
                    tiles = []
                    for h in range(MH):
                        pg = pgp.tile([P, T], F32, tag="pg", name="pg")[:, 0:c]
                        for k in range(KD):
                            nc.tensor.matmul(
                                pg, wg_t[:, k, h * P:(h + 1) * P],
                                xgs[k][:, o:o + c],
                                start=(k == 0), stop=(k == KD - 1))
                        pu = pup.tile([P, T], F32, tag="pu", name="pu")[:, 0:c]
                        for k in range(KD):
                            nc.tensor.matmul(
                                pu, wu_t[:, k, h * P:(h + 1) * P],
                                xgs[k][:, o:o + c],
                                start=(k == 0), stop=(k == KD - 1))
                        sg = silp.tile([P, T], BF16, tag="sg", name="sg")[:, 0:c]
                        nc.scalar.activation(sg, pg, Act.Silu)
                        a_t = a2p.tile([P, MW], BF16, tag="a2")
                        nc.vector.tensor_tensor(a_t[:, 0:c], sg, pu, Alu.mult)
                        tiles.append(a_t)
                    a_tiles[e] = tiles

                def down_t(e, wd_t):
                    tiles = a_tiles.pop(e)
                    for q, (ee, g0, w) in enumerate(chunks):
                        if ee != e:
                            continue
                        s = g0 - int(offs[e])
                        for dh in range(2):
                            yp = pyp.tile([P, DH], F32, tag="yp")
                            for h in range(MH):
                                nc.tensor.matmul(
                                    yp[0:w, :], tiles[h][:, s:s + w],
                                    wd_t[:, h, dh * DH:(dh + 1) * DH],
                                    start=(h == 0), stop=(h == MH - 1))
                            nc.scalar.activation(
                                yts[q][0:w, dh * DH:(dh + 1) * DH], yp[0:w, :],
                                Act.Copy)

                def shared_chunk(c, wsg_t, wsu_t):
                    for half in range(2):
                        j = 2 * c + half
                        pg = pgp.tile([P, T], F32, tag="pg", name="spg")
                        for k in range(KD):
                            nc.tensor.matmul(
                                pg[:], wsg_t[:, k, half * P:(half + 1) * P],
                                xbs[k][:], start=(k == 0), stop=(k == KD - 1))
                        pu = pup.tile([P, T], F32, tag="pu", name="spu")
                        for k in range(KD):
                            nc.tensor.matmul(
                                pu[:], wsu_t[:, k, half * P:(half + 1) * P],
                                xbs[k][:], start=(k == 0), stop=(k == KD - 1))
                        sg = silp.tile([P, T], BF16, tag="sg", name="ssg")
                        nc.scalar.activation(sg[:], pg[:], Act.Silu)
                        nc.vector.tensor_tensor(as_tiles[j][:], sg[:], pu[:],
                                                Alu.mult)

                sched_i = 0  # next shared chunk to load
                done_i = 0   # next shared chunk to compute
                sched_i = 1  # chunk 0 already loading
                for e in range(E):
                    gu_cur, d_cur = gu_next, d_next
                    if e + 1 < E:
                        gu_next = load_gu(e + 1)
                        for _ in range(_SHARED_SCHED[e + 1]):
                            if sched_i < NSC:
                                sh_w.append(load_shared(sched_i))
                                sched_i += 1
                        d_next = load_d(e + 1)
                    if e == E - 2:
                        # prefetch first wsd half on the SP queue right after
                        # the last expert weights; in-order so it cannot be
                        # hoisted ahead of the critical loads
                        wsd_half[0] = []
                        for jc in range(4):
                            t_ = wsdp.tile([P, MH, DH], BF16, tag="wsd")
                            nc.sync.dma_start(t_[:], wsdr[jc, 0])
                            wsd_half[0].append(t_)
                    gate_up(e, gu_cur[0], gu_cur[1])
                    for _ in range(_SHARED_SCHED[e]):
                        if done_i < NSC:
                            shared_chunk(done_i, *sh_w[done_i])
                            sh_w[done_i] = None
                            done_i += 1
                    down_t(e, d_cur)

            # =====================================================
            # Output phase: per (dh, ti): r' and s psums, combine, store
            # =====================================================
            with tc.tile_pool(name=f"psum_s{bi}", bufs=2, space="PSUM") as psp, \
                 tc.tile_pool(name=f"psum_r{bi}", bufs=2, space="PSUM") as prp, \
                 tc.tile_pool(name=f"fin{bi}", bufs=2) as finp, \
                 tc.tile_pool(name=f"outsb{bi}", bufs=4) as outp:
                wsd_half[1] = []
                for jc in range(4):
                    t_ = wsdp.tile([P, MH, DH], BF16, tag="wsd")
                    nc.sync.dma_start(t_[:], wsdr[jc, 1])
                    wsd_half[1].append(t_)
                for dh in range(2):
                    for ti in range(TM):
                        rp = prp.tile([P, DH], F32, tag="rp")
                        for q, (e, g0, w) in enumerate(chunks):
                            nc.tensor.matmul(
                                rp[:], mgs[q][0:w, ti * P:(ti + 1) * P],
                                yts[q][0:w, dh * DH:(dh + 1) * DH],
                                start=(q == 0), stop=(q == NCH - 1))
                        sp = psp.tile([P, DH], F32, tag="sp")
                        for jc in range(4):
                            for j in range(MH):
                                jq = jc * MH + j
                                nc.tensor.matmul(
                                    sp[:],
                                    as_tiles[jq][:, ti * P:(ti + 1) * P],
                                    wsd_half[dh][jc][:, j, :],
                                    start=(jq == 0), stop=(jq == MS - 1))
                        d2 = finp.tile([P, DH], F32, tag="d2")
                        nc.scalar.activation(d2[:], sp[:], Act.Copy,
                                             scale=alph[:, ti:ti + 1])
                        o_ = outp.tile([P, DH], F32, tag="o")
                        nc.vector.tensor_tensor(o_[:], d2[:], rp[:], Alu.add)
                        nc.scalar.dma_start(
                            out[ti * P:(ti + 1) * P, dh * DH:(dh + 1) * DH],
                            o_[:])

        if repeat == 1:
            emit_body(0)
        elif repeat == 2:
            emit_body(0)
            emit_body(1)
        elif repeat % 4 == 0:
            with tc.For_i(0, repeat // 4, 1,
                          staggered_reset=repeat_staggered):
                for bi4 in range(4):
                    emit_body(bi4)
        elif repeat % 2 == 0:
            with tc.For_i(0, repeat // 2, 1,
                          staggered_reset=repeat_staggered):
                emit_body(0)
                emit_body(1)
        else:
            with tc.For_i(0, repeat, 1,
                          staggered_reset=repeat_staggered):
                emit_body(0)

    nc.compile()
    return nc


_NC_CACHE = {}


def _get_program():
    key = _CAPS
    if key not in _NC_CACHE:
        _NC_CACHE[key] = _build_program(caps=_CAPS)
    return _NC_CACHE[key]


def _sigmoid(a):
    return 1.0 / (1.0 + np.exp(-a))


def make_in_maps(x, router_w, w_gate, w_up, w_down, ws_gate, ws_up, ws_down,
                 sg_w, sg_b):
    global _CAPS
    bf = mybir.dt.np(BF16)
    f32 = np.float32
    x2 = np.asarray(x, dtype=f32).reshape(B * S, D)

    logits = x2 @ np.asarray(router_w, f32)
    lmax = logits.max(axis=-1, keepdims=True)
    pr = np.exp(logits - lmax)
    pr /= pr.sum(axis=-1, keepdims=True)
    i1 = np.argmax(logits, axis=-1)
    l2 = np.array(logits)
    l2[np.arange(len(l2)), i1] = -np.inf
    i2 = np.argmax(l2, axis=-1)
    p1 = pr[np.arange(len(pr)), i1]
    p2 = pr[np.arange(len(pr)), i2]
    g1 = p1 / (p1 + p2)
    g2v = p2 / (p1 + p2)
    alpha = _sigmoid(x2 @ np.asarray(sg_w, f32).reshape(D, 1)
                     + np.asarray(sg_b, f32).reshape(1, 1))[:, 0]

    sel = [[[] for _ in range(E)] for _ in range(N_CORES)]
    for c in range(N_CORES):
        li1 = i1[c * T:(c + 1) * T]
        li2 = i2[c * T:(c + 1) * T]
        for t in range(T):
            sel[c][li1[t]].append(t)
            sel[c][li2[t]].append(t)
    caps = tuple(int(max(len(sel[c][e]) for c in range(N_CORES)))
                 for e in range(E))
    _CAPS = caps
    offs, L, chunks = _grid(caps)
    NCH = len(chunks)

    shared = {
        "wg": np.asarray(w_gate, f32).astype(bf),
        "wu": np.asarray(w_up, f32).astype(bf),
        "wd": np.asarray(w_down, f32).astype(bf),
        "wsg": np.asarray(ws_gate, f32).astype(bf),
        "wsu": np.asarray(ws_up, f32).astype(bf),
        "wsd": np.asarray(ws_down, f32).astype(bf),
    }
    in_maps = []
    for c in range(N_CORES):
        xc = x2[c * T:(c + 1) * T, :]
        ac = alpha[c * T:(c + 1) * T]
        gv = {}  # token -> {expert: gate}
        for t in range(T):
            gt = c * T + t
            gv[t] = {i1[gt]: g1[gt], i2[gt]: g2v[gt]}
        xgm = np.zeros((L, D), dtype=f32)
        tok = np.full((P, NCH), -1.0, dtype=f32)
        g2m = np.zeros((P, NCH), dtype=f32)
        for e in range(E):
            idx = sel[c][e]
            o = int(offs[e])
            xgm[o:o + len(idx)] = xc[idx]
        for q, (e, g0, w) in enumerate(chunks):
            idx = sel[c][e]
            s = g0 - int(offs[e])
            part = idx[s:s + w]
            tok[0:len(part), q] = np.asarray(part, f32)
            g2m[0:len(part), q] = [
                gv[t][e] * (1.0 - ac[t]) for t in part]
        am = np.ascontiguousarray(ac.reshape(TM, P).T)  # [P, TM]
        m = dict(shared)
        m["xb"] = np.ascontiguousarray(xc.T).astype(bf)
        m["xg"] = np.ascontiguousarray(xgm.T).astype(bf)
        m["meta"] = np.concatenate([tok, g2m, am], axis=1).astype(f32)
        in_maps.append(m)
    return in_maps


def assemble_out(results):
    rows = [np.asarray(results[c]["out"]) for c in range(N_CORES)]
    return np.concatenate(rows, axis=0).reshape(B, S, D).astype(np.float32)


def kernel(**inputs) -> np.ndarray:
    in_maps = make_in_maps(**inputs)
    nc = _get_program()
    res = run_bass_kernel_spmd(nc, in_maps, list(range(N_CORES)))
    return assemble_out(res.results)


# revision 14
# speedup vs baseline: 1.1580x; 1.1580x over previous
"""MoE (top-2 of 8 experts + shared expert, SwiGLU) on 8 trn2 NeuronCores.

Sharding: data-parallel over tokens (512/core), with SPARSE routed experts.
The host computes the routing (top-2 indices), the renormalized gates, and
the shared-expert mixing factor alpha = sigmoid(x@sg_w+b); it stages, per
core, a gathered activation matrix xg whose columns are the tokens assigned
to each expert (concatenated expert blocks, padded to a shared capacity
table whose total is a multiple of 128) plus a small metadata block
(slot->token index, per-slot combined gate g*(1-alpha), per-token alpha).

Device schedule (per core, per iteration):
- A one-hot scatter matrix M[slot, token] is built on device (iota vs the
  staged slot->token index vector); g*(1-alpha) is folded into M's rows, so
  the scatter matmul directly produces (1-alpha)*routed.  M and the routed
  expert outputs Yt live in DENSE 128-slot tiles that ignore expert
  boundaries (expert spans are split into <=128 "segments" at global tile
  boundaries; the down-projection writes each segment at partition 0 and a
  small SBUF->SBUF DMA shifts it to its packed partition offset).
- Routed experts run per expert (SwiGLU, silu on the Act engine);
  shared-expert gate/up chunks are interleaved BETWEEN routed experts so
  the PE stays busy while routed weights stream from HBM (the routed phase
  alone is DMA-bound).
- Output phase per (dh, ti) tile: scatter matmuls accumulate
  r' = (1-alpha)*routed into one PSUM bank (full 128-deep contractions over
  the dense slot tiles), shared down-projection accumulates s into another;
  out = alpha*s + r' via one Act copy-scale and one DVE add, then the store
  DMA issues immediately.
"""

import numpy as np
from contextlib import ExitStack

import concourse.bass as bass
import concourse.mybir as mybir
import concourse.tile as tile
from concourse import bacc
from concourse.bass_utils import run_bass_kernel_spmd

B, S, D = 4, 1024, 768
E, H, HS = 8, 768, 3072
N_CORES = 8
T = (B * S) // N_CORES  # 512 tokens per core
P = 128
KD = D // P    # 6 k-tiles over d_model
MH = H // P    # 6 m-tiles over expert hidden
MS = HS // P   # 24 m-tiles over shared hidden
TM = T // P    # 4 token tiles
DH = D // 2    # 384, psum-bank-sized half of d_model
NSC = 12       # shared gate/up weight chunks (2 hs-tiles each)
F32 = mybir.dt.float32
BF16 = mybir.dt.bfloat16

Alu = mybir.AluOpType
Act = mybir.ActivationFunctionType
AX = mybir.AxisListType

_DEFAULT_CAPS = (153, 131, 144, 142, 140, 139, 138, 165)
_CAPS = _DEFAULT_CAPS

# after routed expert e, compute this many pending shared gate/up chunks
_SHARED_SCHED = (1, 2, 1, 2, 2, 1, 2, 1)
assert sum(_SHARED_SCHED) == NSC


def _grid(caps):
    """Slot layout: experts packed contiguously; L is a multiple of 128
    (host pads the last expert's capacity). Segments split each expert's
    span at global 128-slot tile boundaries, so every segment lives inside
    one 128-row tile: (expert, global_start, width)."""
    offs = np.concatenate([[0], np.cumsum(caps)]).astype(int)
    L = int(offs[-1])
    assert L % P == 0, caps
    segs = []
    for e in range(E):
        s = int(offs[e])
        end = int(offs[e + 1])
        while s < end:
            w = min(end - s, P - (s % P))
            segs.append((e, s, w))
            s += w
    return offs, L, segs


def _build_program(repeat=1, caps=None, repeat_staggered=False, probe=None,
                   **_ignored):
    caps = tuple(caps) if caps is not None else _CAPS
    offs, L, segs = _grid(caps)
    NSEG = len(segs)
    NT = L // P
    MW = max(caps)
    do_dma = probe != "pe"    # input streaming
    do_pe = probe != "dma"    # compute

    nc = bacc.Bacc("TRN2", target_bir_lowering=False, debug=False,
                   num_devices=N_CORES)

    xb = nc.dram_tensor("xb", [D, T], BF16, kind="ExternalInput")
    xg = nc.dram_tensor("xg", [D, L], BF16, kind="ExternalInput")
    # meta: [tidx (NT) | gate*(1-alpha) (NT) | alpha (TM)], slot-tile-major
    meta = nc.dram_tensor("meta", [P, 2 * NT + TM], F32,
                          kind="ExternalInput")
    wg = nc.dram_tensor("wg", [E, D, H], BF16, kind="ExternalInput")
    wu = nc.dram_tensor("wu", [E, D, H], BF16, kind="ExternalInput")
    wd = nc.dram_tensor("wd", [E, H, D], BF16, kind="ExternalInput")
    wsg = nc.dram_tensor("wsg", [D, HS], BF16, kind="ExternalInput")
    wsu = nc.dram_tensor("wsu", [D, HS], BF16, kind="ExternalInput")
    wsd = nc.dram_tensor("wsd", [HS, D], BF16, kind="ExternalInput")
    out = nc.dram_tensor("out", [T, D], F32, kind="ExternalOutput")

    wgr = wg.rearrange("e (k p) m -> e p k m", p=P)
    wur = wu.rearrange("e (k p) m -> e p k m", p=P)
    wdr = wd.rearrange("e (k p) m -> e p k m", p=P)
    wsgr = wsg.rearrange("(k p) (c m) -> c p k m", p=P, c=NSC)
    wsur = wsu.rearrange("(k p) (c m) -> c p k m", p=P, c=NSC)
    # wsd chunked by (jc: 4 groups of 6 hs-tiles) x (dh: 2 halves of d)
    wsdr = wsd.rearrange("(c j p) (h m) -> c h p j m", p=P, c=4, h=2)

    with tile.TileContext(nc) as tc:
        def emit_body(bi):
          with ExitStack() as ctx:
            # ---- early loads: xg k0 first, then expert-0 gate/up weights
            # (PE's critical path), then the rest of xg ----
            xgp = ctx.enter_context(tc.tile_pool(name=f"xg{bi}", bufs=KD))
            xgs = [xgp.tile([P, L], BF16, tag="xg", name=f"xg{bi}_{k}")
                   for k in range(KD)]
            if do_dma:
                nc.sync.dma_start(xgs[0][:], xg[0:P, :])
            smallp = ctx.enter_context(tc.tile_pool(name=f"small{bi}", bufs=2))
            smf = smallp.tile([P, 2 * NT + TM], F32, tag="smf")
            tidx = smf[:, 0:NT]
            g2 = smf[:, NT:2 * NT]
            alph = smf[:, 2 * NT:2 * NT + TM]

            iota_t = smallp.tile([P, T], F32, tag="iota")
            nc.gpsimd.iota(iota_t[:], [[1, T]], channel_multiplier=0,
                           allow_small_or_imprecise_dtypes=True)

            # ---- long-lived activation storage (dense slot tiles) ----
            mgp = ctx.enter_context(tc.tile_pool(name=f"mg{bi}", bufs=NT))
            ytp = ctx.enter_context(tc.tile_pool(name=f"yt{bi}", bufs=NT))
            asp = ctx.enter_context(tc.tile_pool(name=f"as{bi}", bufs=MS))
            xbp = ctx.enter_context(tc.tile_pool(name=f"xb{bi}", bufs=KD))
            wsdp = ctx.enter_context(tc.tile_pool(name=f"wsd{bi}", bufs=8))

            mgs = [mgp.tile([P, T], BF16, tag="mg", name=f"mg{bi}_{q}")
                   for q in range(NT)]
            yts = [ytp.tile([P, D], BF16, tag="yt", name=f"yt{bi}_{q}")
                   for q in range(NT)]
            as_tiles = [asp.tile([P, T], BF16, tag="as", name=f"as{bi}_{j}")
                        for j in range(MS)]
            wsd_half = {}  # dh -> [4 tiles]

            with tc.tile_pool(name=f"wge{bi}", bufs=2) as wgep, \
                 tc.tile_pool(name=f"wue{bi}", bufs=2) as wuep, \
                 tc.tile_pool(name=f"wde{bi}", bufs=2) as wdep, \
                 tc.tile_pool(name=f"wsh{bi}", bufs=4) as wshp, \
                 tc.tile_pool(name=f"psum_g{bi}", bufs=2, space="PSUM") as pgp, \
                 tc.tile_pool(name=f"psum_u{bi}", bufs=2, space="PSUM") as pup, \
                 tc.tile_pool(name=f"psum_y{bi}", bufs=2, space="PSUM") as pyp, \
                 tc.tile_pool(name=f"sil{bi}", bufs=3) as silp, \
                 tc.tile_pool(name=f"stg{bi}", bufs=3) as stgp, \
                 tc.tile_pool(name=f"a2{bi}", bufs=2 * MH + 2) as a2p:

                def load_gu(e):
                    wg_t = wgep.tile([P, KD, H], BF16, tag="wge")
                    wu_t = wuep.tile([P, KD, H], BF16, tag="wue")
                    if do_dma:
                        nc.sync.dma_start(wg_t[:], wgr[e])
                        nc.sync.dma_start(wu_t[:], wur[e])
                    return wg_t, wu_t

                def load_d(e):
                    wd_t = wdep.tile([P, KD, D], BF16, tag="wde")
                    if do_dma:
                        nc.sync.dma_start(wd_t[:], wdr[e])
                    return wd_t

                def load_shared(c):
                    wsg_t = wshp.tile([P, KD, 2 * P], BF16, tag="wsg")
                    wsu_t = wshp.tile([P, KD, 2 * P], BF16, tag="wsu")
                    if do_dma:
                        nc.sync.dma_start(wsg_t[:], wsgr[c])
                        nc.sync.dma_start(wsu_t[:], wsur[c])
                    return wsg_t, wsu_t

                # initial prefetch: expert 0 g/u right behind xg k0, then
                # the remaining xg k-tiles, meta, xb, shared chunk 0, wd0
                gu_next = load_gu(0)
                if do_dma:
                    for k in range(1, KD):
                        nc.sync.dma_start(xgs[k][:], xg[k * P:(k + 1) * P, :])
                nc.sync.dma_start(smf[:], meta[:, :])
                xbs = []
                for k in range(KD):
                    t_ = xbp.tile([P, T], BF16, tag="xb")
                    if do_dma:
                        nc.sync.dma_start(t_[:], xb[k * P:(k + 1) * P, :])
                    xbs.append(t_)
                sh_w = [load_shared(0)]
                d_next = load_d(0)

                # ---- scatter matrix build (DVE/Act, runs during DMA) ----
                for tn in range(NT):
                    mf = silp.tile([P, T], BF16, tag="mf", name="mf")
                    nc.vector.tensor_scalar(mf[:], iota_t[:],
                                            tidx[:, tn:tn + 1], None,
                                            Alu.is_equal)
                    nc.scalar.activation(mgs[tn][:], mf[:],
                                         Act.Copy, scale=g2[:, tn:tn + 1])

                a_tiles = {}

                def gate_up(e, wg_t, wu_t):
                    c = caps[e]
                    o = int(offs[e])
                    tiles = []
                    for h in range(MH):
                        pg = pgp.tile([P, T], F32, tag="pg", name="pg")[:, 0:c]
                        for k in range(KD):
                            nc.tensor.matmul(
                                pg, wg_t[:, k, h * P:(h + 1) * P],
                                xgs[k][:, o:o + c],
                                start=(k == 0), stop=(k == KD - 1))
                        pu = pup.tile([P, T], F32, tag="pu", name="pu")[:, 0:c]
                        for k in range(KD):
                            nc.tensor.matmul(
                                pu, wu_t[:, k, h * P:(h + 1) * P],
                                xgs[k][:, o:o + c],
                                start=(k == 0), stop=(k == KD - 1))
                        sg = silp.tile([P, T], BF16, tag="sg", name="sg")[:, 0:c]
                        nc.scalar.activation(sg, pg, Act.Silu)
                        a_t = a2p.tile([P, MW], BF16, tag="a2")
                        nc.vector.tensor_tensor(a_t[:, 0:c], sg, pu, Alu.mult)
                        tiles.append(a_t)
                    a_tiles[e] = tiles

                def down_t(e, wd_t):
                    tiles = a_tiles.pop(e)
                    for q, (ee, g0, w) in enumerate(segs):
                        if ee != e:
                            continue
                        tn, o = g0 // P, g0 % P
                        s = g0 - int(offs[e])
                        stg = stgp.tile([P, D], BF16, tag="stg", name="stg")
                        for dh in range(2):
                            yp = pyp.tile([P, DH], F32, tag="yp")
                            for h in range(MH):
                                nc.tensor.matmul(
                                    yp[0:w, :], tiles[h][:, s:s + w],
                                    wd_t[:, h, dh * DH:(dh + 1) * DH],
                                    start=(h == 0), stop=(h == MH - 1))
                            nc.scalar.activation(
                                stg[0:w, dh * DH:(dh + 1) * DH], yp[0:w, :],
                                Act.Copy)
                        # partition-shift into the packed slot tile
                        nc.scalar.dma_start(yts[tn][o:o + w, :], stg[0:w, :])

                def shared_chunk(c, wsg_t, wsu_t):
                    for half in range(2):
                        j = 2 * c + half
                        pg = pgp.tile([P, T], F32, tag="pg", name="spg")
                        for k in range(KD):
                            nc.tensor.matmul(
                                pg[:], wsg_t[:, k, half * P:(half + 1) * P],
                                xbs[k][:], start=(k == 0), stop=(k == KD - 1))
                        pu = pup.tile([P, T], F32, tag="pu", name="spu")
                        for k in range(KD):
                            nc.tensor.matmul(
                                pu[:], wsu_t[:, k, half * P:(half + 1) * P],
                                xbs[k][:], start=(k == 0), stop=(k == KD - 1))
                        sg = silp.tile([P, T], BF16, tag="sg", name="ssg")
                        nc.scalar.activation(sg[:], pg[:], Act.Silu)
                        nc.vector.tensor_tensor(as_tiles[j][:], sg[:], pu[:],
                                                Alu.mult)

                done_i = 0   # next shared chunk to compute
                sched_i = 1  # chunk 0 already loading
                for e in range(E):
                    gu_cur, d_cur = gu_next, d_next
                    if e + 1 < E:
                        gu_next = load_gu(e + 1)
                        for _ in range(_SHARED_SCHED[e + 1]):
                            if sched_i < NSC:
                                sh_w.append(load_shared(sched_i))
                                sched_i += 1
                        d_next = load_d(e + 1)
                    if e >= E - 2:
                        # prefetch a wsd half per tail expert on the SP queue
                        half = e - (E - 2)
                        wsd_half[half] = []
                        for jc in range(4):
                            t_ = wsdp.tile([P, MH, DH], BF16, tag="wsd")
                            if do_dma:
                                nc.sync.dma_start(t_[:], wsdr[jc, half])
                            wsd_half[half].append(t_)
                    if do_pe:
                        gate_up(e, gu_cur[0], gu_cur[1])
                    for _ in range(_SHARED_SCHED[e]):
                        if done_i < NSC:
                            if do_pe:
                                shared_chunk(done_i, *sh_w[done_i])
                            sh_w[done_i] = None
                            done_i += 1
                    if do_pe:
                        down_t(e, d_cur)

            # =====================================================
            # Output phase: per (dh, ti): r' and s psums, combine, store
            # =====================================================
            with tc.tile_pool(name=f"psum_s{bi}", bufs=2, space="PSUM") as psp, \
                 tc.tile_pool(name=f"psum_r{bi}", bufs=2, space="PSUM") as prp, \
                 tc.tile_pool(name=f"fin{bi}", bufs=2) as finp, \
                 tc.tile_pool(name=f"outsb{bi}", bufs=4) as outp:
                for dh in range(2):
                    for ti in range(TM):
                        o_ = outp.tile([P, DH], F32, tag="o")
                        if do_pe:
                            rp = prp.tile([P, DH], F32, tag="rp")
                            for tn in range(NT):
                                nc.tensor.matmul(
                                    rp[:], mgs[tn][:, ti * P:(ti + 1) * P],
                                    yts[tn][:, dh * DH:(dh + 1) * DH],
                                    start=(tn == 0), stop=(tn == NT - 1))
                            sp = psp.tile([P, DH], F32, tag="sp")
                            for jc in range(4):
                                for j in range(MH):
                                    jq = jc * MH + j
                                    nc.tensor.matmul(
                                        sp[:],
                                        as_tiles[jq][:, ti * P:(ti + 1) * P],
                                        wsd_half[dh][jc][:, j, :],
                                        start=(jq == 0), stop=(jq == MS - 1))
                            d2 = finp.tile([P, DH], F32, tag="d2")
                            nc.scalar.activation(d2[:], sp[:], Act.Copy,
                                                 scale=alph[:, ti:ti + 1])
                            nc.vector.tensor_tensor(o_[:], d2[:], rp[:],
                                                    Alu.add)
                        else:
                            nc.scalar.activation(o_[:], iota_t[:, 0:DH],
                                                 Act.Copy)
                        nc.scalar.dma_start(
                            out[ti * P:(ti + 1) * P, dh * DH:(dh + 1) * DH],
                            o_[:])

        if repeat == 1:
            emit_body(0)
        elif repeat == 2:
            emit_body(0)
            emit_body(1)
        elif repeat % 4 == 0:
            with tc.For_i(0, repeat // 4, 1,
                          staggered_reset=repeat_staggered):
                for bi4 in range(4):
                    emit_body(bi4)
        elif repeat % 2 == 0:
            with tc.For_i(0, repeat // 2, 1,
                          staggered_reset=repeat_staggered):
                emit_body(0)
                emit_body(1)
        else:
            with tc.For_i(0, repeat, 1,
                          staggered_reset=repeat_staggered):
                emit_body(0)

    nc.compile()
    return nc


_NC_CACHE = {}


def _get_program():
    key = _CAPS
    if key not in _NC_CACHE:
        _NC_CACHE[key] = _build_program(caps=_CAPS)
    return _NC_CACHE[key]


def _sigmoid(a):
    return 1.0 / (1.0 + np.exp(-a))


def make_in_maps(x, router_w, w_gate, w_up, w_down, ws_gate, ws_up, ws_down,
                 sg_w, sg_b):
    global _CAPS
    bf = mybir.dt.np(BF16)
    f32 = np.float32
    x2 = np.asarray(x, dtype=f32).reshape(B * S, D)

    logits = x2 @ np.asarray(router_w, f32)
    lmax = logits.max(axis=-1, keepdims=True)
    pr = np.exp(logits - lmax)
    pr /= pr.sum(axis=-1, keepdims=True)
    i1 = np.argmax(logits, axis=-1)
    l2 = np.array(logits)
    l2[np.arange(len(l2)), i1] = -np.inf
    i2 = np.argmax(l2, axis=-1)
    p1 = pr[np.arange(len(pr)), i1]
    p2 = pr[np.arange(len(pr)), i2]
    g1 = p1 / (p1 + p2)
    g2v = p2 / (p1 + p2)
    alpha = _sigmoid(x2 @ np.asarray(sg_w, f32).reshape(D, 1)
                     + np.asarray(sg_b, f32).reshape(1, 1))[:, 0]

    sel = [[[] for _ in range(E)] for _ in range(N_CORES)]
    for c in range(N_CORES):
        li1 = i1[c * T:(c + 1) * T]
        li2 = i2[c * T:(c + 1) * T]
        for t in range(T):
            sel[c][li1[t]].append(t)
            sel[c][li2[t]].append(t)
    caps = [int(max(len(sel[c][e]) for c in range(N_CORES)))
            for e in range(E)]
    # pad the last expert's capacity so the slot count is a multiple of 128
    Lr = sum(caps)
    caps[E - 1] += (P - Lr % P) % P
    caps = tuple(caps)
    _CAPS = caps
    offs, L, segs = _grid(caps)
    NSEG = len(segs)

    shared = {
        "wg": np.asarray(w_gate, f32).astype(bf),
        "wu": np.asarray(w_up, f32).astype(bf),
        "wd": np.asarray(w_down, f32).astype(bf),
        "wsg": np.asarray(ws_gate, f32).astype(bf),
        "wsu": np.asarray(ws_up, f32).astype(bf),
        "wsd": np.asarray(ws_down, f32).astype(bf),
    }
    in_maps = []
    for c in range(N_CORES):
        xc = x2[c * T:(c + 1) * T, :]
        ac = alpha[c * T:(c + 1) * T]
        gv = {}  # token -> {expert: gate}
        for t in range(T):
            gt = c * T + t
            gv[t] = {i1[gt]: g1[gt], i2[gt]: g2v[gt]}
        xgm = np.zeros((L, D), dtype=f32)
        tokf = np.full((L,), -1.0, dtype=f32)
        g2f = np.zeros((L,), dtype=f32)
        for e in range(E):
            idx = sel[c][e]
            o = int(offs[e])
            xgm[o:o + len(idx)] = xc[idx]
            tokf[o:o + len(idx)] = np.asarray(idx, f32)
            g2f[o:o + len(idx)] = [gv[t][e] * (1.0 - ac[t]) for t in idx]
        # slot-tile-major: column tn holds rows for slots [tn*P, (tn+1)*P)
        tok = np.ascontiguousarray(tokf.reshape(L // P, P).T)
        g2m = np.ascontiguousarray(g2f.reshape(L // P, P).T)
        am = np.ascontiguousarray(ac.reshape(TM, P).T)  # [P, TM]
        m = dict(shared)
        m["xb"] = np.ascontiguousarray(xc.T).astype(bf)
        m["xg"] = np.ascontiguousarray(xgm.T).astype(bf)
        m["meta"] = np.concatenate([tok, g2m, am], axis=1).astype(f32)
        in_maps.append(m)
    return in_maps


def assemble_out(results):
    rows = [np.asarray(results[c]["out"]) for c in range(N_CORES)]
    return np.concatenate(rows, axis=0).reshape(B, S, D).astype(np.float32)


def kernel(**inputs) -> np.ndarray:
    in_maps = make_in_maps(**inputs)
    nc = _get_program()
    res = run_bass_kernel_spmd(nc, in_maps, list(range(N_CORES)))
    return assemble_out(res.results)
